# revision 7
# baseline (speedup 1.0000x reference)
"""AdditiveAttention Trainium2 kernel (8 NeuronCores, data-parallel over batch).

Reference computation (B=32, T=2048, D=U=512, fp32):
    query = values[:, -1] @ W2_w + W2_b                     # [B, U]
    keys  = values @ W1_w + W1_b                            # [B, T, U]
    score = tanh(keys + query[:, None, :]) @ V_w + V_b      # [B, T, 1]
    attn  = softmax(score, axis=1)
    out   = sum(attn * values, axis=1)                      # [B, D]

Sharding: data-parallel over B (4 batches per core), weights replicated,
no collectives.  Compute in bf16 on the TensorEngine (fp32 accumulate in
PSUM); validated end-to-end rel-err ~3e-3 vs the fp32 reference.

Per-core dataflow (per batch b):
  - valuesT  (D on partitions) via xbar DMA-transpose  -> keys GEMM operand
  - keysT[u-chunk] = sum_c W1_blk(c,u).T @ valuesT(c)  -> PSUM [128, 512]
  - tanh fused on ACT: tanh(keysT + (query+W1_b+W2_b)[u] per-partition bias)
  - score chunk [1, 512] = sum_u V(u).T @ tanh_keysT(u)  (PE, M=1)
  - softmax: exp on ACT with accum_out=Z (no max-subtraction: |score|<3)
  - attn transposed to T-on-partitions via PE transpose
  - out[b] = sum_k attn(k).T @ values_nat(k)  (PE, M=1) scaled by 1/Z
V_b drops out of softmax (constant shift).
"""

import os
from contextlib import ExitStack

import numpy as np
import ml_dtypes

import concourse.bass as bass
import concourse.tile as tile
from concourse import bacc, mybir
from concourse.bass_utils import run_bass_kernel_spmd

BF16 = ml_dtypes.bfloat16

B, T, D, U = 32, 2048, 512, 512
NCORES = 8
BSH = B // NCORES          # 4 batches per core
P = 128
DC = D // P                # 4 chunks of D
UC = U // P                # 4 chunks of U
TS = 512                   # T tile (moving free dim) for keys GEMM
TN = T // TS               # 4
TK = T // P                # 16 chunks of T for transposes / weighted sum

_GRAPH = None


def _build_graph():
    nc = bacc.Bacc("TRN2", target_bir_lowering=False, debug=False)
    bf = mybir.dt.bfloat16
    f32 = mybir.dt.float32

    vals = nc.declare_dram_parameter("vals", [BSH, T, D], bf, isOutput=False)
    w1 = nc.declare_dram_parameter("w1", [D, U], bf, isOutput=False)
    w2 = nc.declare_dram_parameter("w2", [D, U], bf, isOutput=False)
    vw = nc.declare_dram_parameter("vw", [U, 1], bf, isOutput=False)
    bsum = nc.declare_dram_parameter("bsum", [U, 1], f32, isOutput=False)
    ident = nc.declare_dram_parameter("ident", [P, P], bf, isOutput=False)
    out_ext = nc.declare_dram_parameter("out", [BSH, D], f32, isOutput=True)

    Tanh = mybir.ActivationFunctionType.Tanh
    Exp = mybir.ActivationFunctionType.Exp

    with tile.TileContext(nc) as tc, ExitStack() as ctx:
        const = ctx.enter_context(tc.tile_pool(name="const", bufs=1))
        valt_pool = ctx.enter_context(tc.tile_pool(name="valt", bufs=2))
        nat_pool = ctx.enter_context(tc.tile_pool(name="nat", bufs=BSH))
        qb_pool = ctx.enter_context(tc.tile_pool(name="qb", bufs=2))
        tk_pool = ctx.enter_context(tc.tile_pool(name="tk", bufs=3))
        sm_pool = ctx.enter_context(tc.tile_pool(name="sm", bufs=1))
        kps = ctx.enter_context(tc.tile_pool(name="kps", bufs=2, space="PSUM"))
        sps = ctx.enter_context(tc.tile_pool(name="sps", bufs=2, space="PSUM"))
        aps = ctx.enter_context(tc.tile_pool(name="aps", bufs=2, space="PSUM"))

        # ---- constants -------------------------------------------------
        # w1_sb[p, c, u] = W1[c*128 + p, u]  (lhsT blocks: K=D-chunk on parts)
        w1_sb = const.tile([P, DC, U], bf)
        nc.sync.dma_start(w1_sb[:], w1.ap().rearrange("(c p) u -> p c u", p=P))
        w2_sb = const.tile([P, DC, U], bf)
        nc.sync.dma_start(w2_sb[:], w2.ap().rearrange("(c p) u -> p c u", p=P))
        v_sb = const.tile([P, UC], bf)
        nc.sync.dma_start(v_sb[:], vw.ap().rearrange("(c p) one -> p (c one)", p=P))
        bsum_sb = const.tile([P, UC], mybir.dt.float32)
        nc.sync.dma_start(bsum_sb[:], bsum.ap().rearrange("(c p) one -> p (c one)", p=P))
        ident_sb = const.tile([P, P], bf)
        nc.sync.dma_start(ident_sb[:], ident.ap())

        # ---- values loads ---------------------------------------------
        # natural layout for the weighted sum: nat[b][p, n, d] = vals[b, n*128+p, d]
        nats = []
        for b in range(BSH):
            nat_b = nat_pool.tile([P, TK, D], bf, tag="nat")
            nc.sync.dma_start(
                nat_b[:], vals.ap()[b].rearrange("(n p) d -> p n d", p=P)
            )
            nats.append(nat_b)

        # per-batch score rows live at partition 0 (compute engines may only
        # address partition starts 0/32/64/96); DMA assembles the [4, T] block
        score_rows = [
            sm_pool.tile([1, T], mybir.dt.float32, name=f"srow{b}", tag=f"srow{b}")
            for b in range(BSH)
        ]
        score_sb = sm_pool.tile([BSH, T], mybir.dt.float32)

        for b in range(BSH):
            # transposed layout: valt[p, c, t] = vals[b, t, c*128+p]
            valt = valt_pool.tile([P, DC, T], bf, tag="valt")
            for c in range(DC):
                nc.sync.dma_start(
                    valt[:, c], vals.ap()[b, :, c * P : (c + 1) * P], transpose=True
                )

            # query_b = vals[b, -1] @ W2 (+ W1_b + W2_b) -> per-partition bias
            qb = qb_pool.tile([P, UC], mybir.dt.float32, tag="qb")
            for u in range(UC):
                qp = kps.tile([P, TS], mybir.dt.float32, tag="kps")
                for c in range(DC):
                    nc.tensor.matmul(
                        qp[:, 0:1],
                        w2_sb[:, c, u * P : (u + 1) * P],
                        valt[:, c, T - 1 : T],
                        start=(c == 0),
                        stop=(c == DC - 1),
                    )
                nc.vector.tensor_scalar_add(
                    qb[:, u : u + 1], qp[:, 0:1], bsum_sb[:, u : u + 1]
                )

            # keys -> tanh -> score
            for s in range(TN):
                sp = sps.tile([1, TS], mybir.dt.float32, tag="sps")
                for u in range(UC):
                    kp = kps.tile([P, TS], mybir.dt.float32, tag="kps")
                    for c in range(DC):
                        nc.tensor.matmul(
                            kp[:],
                            w1_sb[:, c, u * P : (u + 1) * P],
                            valt[:, c, s * TS : (s + 1) * TS],
                            start=(c == 0),
                            stop=(c == DC - 1),
                        )
                    tkt = tk_pool.tile([P, TS], bf, tag="tk")
                    nc.scalar.activation(tkt[:], kp[:], Tanh, bias=qb[:, u : u + 1])
                    nc.tensor.matmul(
                        sp[:],
                        v_sb[:, u : u + 1],
                        tkt[:],
                        start=(u == 0),
                        stop=(u == UC - 1),
                    )
                nc.any.tensor_copy(
                    score_rows[b][0:1, s * TS : (s + 1) * TS], sp[:]
                )

        # assemble the 4 score rows onto partitions 0..3 (DMA may write any
        # partition; compute engines may not)
        for b in range(BSH):
            nc.sync.dma_start(score_sb[b : b + 1, :], score_rows[b][:])

        # ---- softmax over T (per batch row) ---------------------------
        e_sb = sm_pool.tile([BSH, T], mybir.dt.bfloat16)
        zrow = sm_pool.tile([BSH, 1], mybir.dt.float32)
        nc.scalar.activation(e_sb[:], score_sb[:], Exp, accum_out=zrow[:])
        rrow = sm_pool.tile([BSH, 1], mybir.dt.float32)
        nc.vector.reciprocal(rrow[:], zrow[:])
        # normalize now (per-partition 1/Z) so the weighted sum needs no tail fixup
        attn_sb = sm_pool.tile([BSH, T], mybir.dt.bfloat16)
        nc.vector.tensor_scalar_mul(attn_sb[:], e_sb[:], rrow[:, 0:1])

        # ---- transpose attn rows to T-on-partitions -------------------
        at_sb = sm_pool.tile([P, TK, BSH], mybir.dt.bfloat16)
        for k in range(TK):
            ap_t = aps.tile([P, BSH], mybir.dt.bfloat16, tag="aps")
            nc.tensor.transpose(
                ap_t[:], attn_sb[:, k * P : (k + 1) * P], ident_sb[0:BSH, 0:BSH]
            )
            nc.vector.tensor_copy(at_sb[:, k, :], ap_t[:])

        # ---- weighted sum over T --------------------------------------
        for b in range(BSH):
            op = sps.tile([1, D], mybir.dt.float32, tag="sps")
            for k in range(TK):
                nc.tensor.matmul(
                    op[:],
                    at_sb[:, k, b : b + 1],
                    nats[b][:, k],
                    start=(k == 0),
                    stop=(k == TK - 1),
                )
            ob = sm_pool.tile([1, D], mybir.dt.float32, name=f"ob{b}", tag=f"ob{b}")
            nc.vector.tensor_copy(ob[:], op[:])
            nc.sync.dma_start(out_ext.ap()[b : b + 1, :], ob[:])

    nc.finalize()
    return nc


def _get_graph():
    global _GRAPH
    if _GRAPH is None:
        _GRAPH = _build_graph()
    return _GRAPH


def _make_in_maps(values, W1_w, W1_b, W2_w, W2_b, V_w, V_b):
    vals_bf = np.ascontiguousarray(values).astype(BF16)
    w1_bf = np.ascontiguousarray(W1_w).astype(BF16)
    w2_bf = np.ascontiguousarray(W2_w).astype(BF16)
    v_bf = np.ascontiguousarray(V_w).astype(BF16)
    bsum = (
        np.asarray(W1_b, np.float32) + np.asarray(W2_b, np.float32)
    ).reshape(U, 1)
    ident = np.eye(P, dtype=BF16)

    in_maps = []
    for core in range(NCORES):
        sl = slice(core * BSH, (core + 1) * BSH)
        in_maps.append(
            {
                "vals": vals_bf[sl],
                "w1": w1_bf,
                "w2": w2_bf,
                "vw": v_bf,
                "bsum": bsum,
                "ident": ident,
            }
        )
    return in_maps


def run(inputs, trace=False, **kw):
    """Build + run on 8 cores; returns (full_output, BassKernelResults)."""
    nc = _get_graph()
    in_maps = _make_in_maps(**inputs)
    res = run_bass_kernel_spmd(
        nc, in_maps, core_ids=list(range(NCORES)), trace=trace, **kw
    )
    out = np.concatenate([np.asarray(r["out"]) for r in res.results], axis=0)
    return out.astype(np.float32), res


def kernel(**inputs) -> np.ndarray:
    out, _ = run(inputs)
    return out
